# revision 13
# baseline (speedup 1.0000x reference)
"""Trainium2 Bass kernel for BlockUncertaintyTracker (segment_reduce).

Computes, per 4x4 block of a [16,1,2048,2048] image batch:
  - mean over the 16 block elements, averaged over batch
  - 0.9-quantile (= 0.5*(2nd largest + 3rd largest)), averaged over batch
  - EMA update of both stats, then broadcasts the ratio back to full shape.

Sharding: spatial over H across 8 cores (256 image rows / 64 block rows per
core). Every core sees all 16 batch elements for its rows, so no collectives
are needed.

Row-major subchunk pipeline (8 subchunks/core, each = 8 block rows x all 16
batches, partition p = (b,i)): output writes of subchunk g overlap input
reads of g+1, so the DMA queues stream reads+writes continuously instead of
serializing a write-only tail.

Engine split per subchunk (4 row-phase tiles R_r [128,2048] f32):
  - ScalarE: cast -> fp16, even/odd de-interleaves, table Reciprocal,
    most of the output expand.
  - VectorE: vertical sorted-3 + a-level and b-level merges (all fp16
    contiguous step-1 = 2x mode) + ratio multiply.
  - GpSimd:  ALU unused (shares SBUF ports with DVE and its copies run
    ~3us; measured to degrade DVE to 1x). Only issues input-load DMAs so
    descriptor generation spreads across sequencers.
  - TensorE: block sums via 16 strided-rhs fp16 matmuls (1 cyc/row vs 4 for
    f32) + quantile + EMA terms accumulated in PSUM. The EMA buffers enter
    PSUM through two tiny matmuls of host-precomputed (99*ema + 100*eps)
    fp16 tensors: scaling num and den by 100 leaves the ratio unchanged and
    keeps every lhsT weight fp16-normal.
  - Output: u expanded x4 along columns on ScalarE, written as 4
    full-partition 1 MiB DMAs per subchunk (4 batches each).
"""

import os

import numpy as np

# ---- problem constants (hardcoded; kernel.py must be self-contained) ----
B = 16          # batch
H = 2048
W = 2048
BS = 4          # block size
NCORES = 8
HS = H // NCORES            # 256 rows per core per batch
NBH = HS // BS              # 64 block rows per core
NBW = W // BS               # 512 block cols
ROWS = B * HS               # 4096 rows in a per-core slab
NSUB = 8                    # subchunks per core
IB = NBH // NSUB            # 8 block rows per subchunk
DECAY = 0.99
ALPHA = 0.1
EPS = 1e-5
# num' = 100*num, den' = 100*den; u = num'/den' unchanged.
W_SUM = 1.0 / (BS * BS * B)       # 100*(1-decay)*mean weight = plain mean
W_Q = 0.5 / B                     # 100*(1-decay)*0.5/B = 0.03125 (fp16 exact)
EMA_SCALE = 100.0 * DECAY         # 99.0
EMA_BIAS = 100.0 * EPS            # 1e-3 (den only)

_CACHE = {}


def _split_multi_waits(nc):
    """This walrus build encodes at most ONE sync wait per instruction.
    Tile attaches several. Hoist excess waits onto same-engine NOPs placed
    immediately before the owning instruction (same engine stream => same
    semantics)."""
    import concourse.mybir as mybir

    plans = []  # (inst_name, extra_waits)
    for f in nc.m.functions:
        for bb in f.blocks:
            for inst in bb.instructions:
                si = getattr(inst, "sync_info", None)
                waits = list(si.on_wait) if (si and si.on_wait) else []
                if len(waits) > 1:
                    si.on_wait = [waits[-1]]
                    plans.append((inst.name, waits[:-1]))

    if not plans:
        return

    nop_for = {}
    stray = set()
    for iname, extra in plans:
        nops = []
        for w in extra:
            nop = nc.engines[nc.inst_map[iname].engine].nop(nofuse=True).ins
            nop.sync_info = mybir.SyncInfo(on_wait=[w], on_update=[])
            nops.append(nop)
            stray.add(nop.name)
        nop_for[iname] = nops

    for f in nc.m.functions:
        for bb in f.blocks:
            out = []
            changed = False
            for inst in bb.instructions:
                if inst.name in stray:
                    changed = True
                    continue
                if inst.name in nop_for:
                    out.extend(nop_for[inst.name])
                    changed = True
                out.append(inst)
            if changed:
                bb.instructions = out


def _build():
    """Builds the single-core Bass program (SPMD across 8 cores)."""
    from contextlib import ExitStack

    import concourse.bass as bass
    import concourse.mybir as mybir
    import concourse.tile as tile

    f32 = mybir.dt.float32
    f16 = mybir.dt.float16
    MAX = mybir.AluOpType.max
    MIN = mybir.AluOpType.min
    MULT = mybir.AluOpType.mult

    nc = bass.Bass("TRN2", target_bir_lowering=False, debug=False)

    x = nc.dram_tensor("x", [ROWS, W], f32, kind="ExternalInput").ap()
    # host-precomputed 99*ema_errors + 1e-3 / 99*ema_quantile, fp16
    ee = nc.dram_tensor("ee", [NBH, NBW], f16, kind="ExternalInput").ap()
    eq = nc.dram_tensor("eq", [NBH, NBW], f16, kind="ExternalInput").ap()
    # wsum[p, m] = W_SUM  iff p%8 == (m%32)//4   (batch fold + m duplication)
    # wq  [p, m] = W_Q    same mask
    # wema[p, m] = 1.0    iff p%8 == (m%32)//4   (p in local subchunk coords)
    wsum = nc.dram_tensor("wsum", [128, 128], f16, kind="ExternalInput").ap()
    wq = nc.dram_tensor("wq", [128, 128], f16, kind="ExternalInput").ap()
    wema = nc.dram_tensor("wema", [IB, 128], f16, kind="ExternalInput").ap()
    y = nc.dram_tensor("y", [ROWS, W], f32, kind="ExternalOutput").ap()

    # input row = b*256 + g*32 + i*4 + r; per (g, r): [16, 8, 2048] zips
    # against [128=(b,i), 2048]; 8 KiB lines sustain full DMA bandwidth
    xr = x.rearrange("(b g i r) w -> g r b i w", b=B, g=NSUB, i=IB, r=BS)
    # output row = (t*4+beta)*256 + g*32 + i*4 + r; per (g, t): [4, 8, 4, 2048]
    # zips against 128 partitions as (beta, i, r)
    y6 = y.rearrange(
        "(t beta g i r) w -> g t beta i r w", t=4, beta=4, g=NSUB, i=IB, r=BS
    )

    with tile.TileContext(nc) as tc, ExitStack() as ctx:
        pool = ctx.enter_context(tc.tile_pool(name="work", bufs=1))
        ppool = ctx.enter_context(tc.tile_pool(name="acc", bufs=1, space="PSUM"))

        wsum_sb = pool.tile([128, 128], f16, tag="wsum")
        nc.sync.dma_start(wsum_sb[:, :], wsum)
        wq_sb = pool.tile([128, 128], f16, tag="wq")
        nc.sync.dma_start(wq_sb[:, :], wq)
        wema_sb = pool.tile([IB, 128], f16, tag="wema")
        nc.sync.dma_start(wema_sb[:, :], wema)

        def tt(dst, a, bb, op):
            nc.vector.tensor_tensor(dst, a, bb, op)

        def act_recip(out, in_):
            # table-based reciprocal on the Activation engine (~2e-5 max
            # err on our range; bass's wrapper over-conservatively bans it)
            nc.scalar.add_instruction(
                mybir.InstActivation(
                    name=nc.get_next_instruction_name(),
                    func=mybir.ActivationFunctionType.Reciprocal,
                    ins=[
                        nc.scalar.lower_ap(in_),
                        mybir.ImmediateValue(dtype=f32, value=0.0),
                        mybir.ImmediateValue(dtype=f32, value=1.0),
                        mybir.ImmediateValue(dtype=f32, value=0.0),
                    ],
                    outs=[nc.scalar.lower_ap(out)],
                )
            )

        u_tiles = {}

        def emit_compute(g):
            # ---- load: 4 DMAs into one [128, 4*2048] tile ----
            rt = pool.tile([128, BS * W], f32, tag="raw", bufs=2, name=f"rt_{g}")
            rtv = rt.rearrange("p (r w) -> p r w", r=BS)
            for r in range(BS):
                nc.gpsimd.dma_start(rtv[:, r, :], xr[g, r])

            # ---- PSUM accumulators ----
            psum_s = ppool.tile([128, NBW], f32, tag="ps", bufs=2, name=f"ps_{g}")
            psum_q = ppool.tile([128, NBW], f32, tag="pq", bufs=2, name=f"pq_{g}")

            # EMA term first (opens the accumulation group)
            ee_g = pool.tile([IB, NBW], f16, tag="eesb", bufs=2, name=f"ee_{g}")
            nc.gpsimd.dma_start(ee_g[:, :], ee[g * IB : (g + 1) * IB, :])
            eq_g = pool.tile([IB, NBW], f16, tag="eqsb", bufs=2, name=f"eq_{g}")
            nc.gpsimd.dma_start(eq_g[:, :], eq[g * IB : (g + 1) * IB, :])
            nc.tensor.matmul(
                psum_s[:, :], lhsT=wema_sb[:, :], rhs=ee_g[:, :],
                start=True, stop=False,
            )

            # ---- fused cast + even/odd deinterleave (scalar) ----
            # bte[p, (r, j, ce)] = f16(x[p, r, 4j + 2ce]);  bto: odd cols
            HW2 = W // 2
            rt4 = rt.rearrange("p (r j two) -> p r j two", r=BS, two=2)
            bte = pool.tile([128, BS * HW2], f16, tag="bte", bufs=2, name=f"bte_{g}")
            bto = pool.tile([128, BS * HW2], f16, tag="bto", bufs=2, name=f"bto_{g}")
            for r in range(BS):
                nc.scalar.copy(bte[:, r * HW2 : (r + 1) * HW2], rt4[:, r, :, 0])
                nc.scalar.copy(bto[:, r * HW2 : (r + 1) * HW2], rt4[:, r, :, 1])
            btev = bte.rearrange("p (r w) -> p r w", r=BS)
            btov = bto.rearrange("p (r w) -> p r w", r=BS)

            # ---- block sums: 16 strided fp16 matmuls ----
            bqe = bte.rearrange("p (r j c) -> p r j c", r=BS, c=2)
            bqo = bto.rearrange("p (r j c) -> p r j c", r=BS, c=2)
            for r in range(BS):
                for bq, c in ((bqe, 0), (bqe, 1), (bqo, 0), (bqo, 1)):
                    nc.tensor.matmul(
                        psum_s[:, :], lhsT=wsum_sb[:, :], rhs=bq[:, r, :, c],
                        start=False,
                        stop=(r == BS - 1 and bq is bqo and c == 1),
                    )

            # ---- vertical sorted-3 (DVE, fp16 2x) on e/o planes ----
            planes = {}
            for par, bv in (("e", btev), ("o", btov)):
                b0, b1, b2_, b3 = (bv[:, r, :] for r in range(BS))
                v1 = pool.tile([128, HW2], f16, tag="big", bufs=12, name=f"v1{par}_{g}")
                tt(v1[:, :], b0, b1, MAX)
                w1v = pool.tile([128, HW2], f16, tag="big", bufs=12, name=f"w1v{par}_{g}")
                tt(w1v[:, :], b0, b1, MIN)
                v2 = pool.tile([128, HW2], f16, tag="big", bufs=12, name=f"v2{par}_{g}")
                tt(v2[:, :], b2_, b3, MAX)
                w2v = pool.tile([128, HW2], f16, tag="big", bufs=12, name=f"w2v{par}_{g}")
                tt(w2v[:, :], b2_, b3, MIN)
                m = pool.tile([128, HW2], f16, tag="big", bufs=12, name=f"m{par}_{g}")
                tt(m[:, :], v1[:, :], v2[:, :], MAX)
                t1 = pool.tile([128, HW2], f16, tag="big", bufs=12, name=f"t1{par}_{g}")
                tt(t1[:, :], v1[:, :], v2[:, :], MIN)
                t2 = pool.tile([128, HW2], f16, tag="big", bufs=12, name=f"t2{par}_{g}")
                tt(t2[:, :], w1v[:, :], w2v[:, :], MAX)
                s2 = pool.tile([128, HW2], f16, tag="big", bufs=12, name=f"s2{par}_{g}")
                tt(s2[:, :], t1[:, :], t2[:, :], MAX)
                t3 = pool.tile([128, HW2], f16, tag="big", bufs=12, name=f"t3{par}_{g}")
                tt(t3[:, :], t1[:, :], t2[:, :], MIN)
                planes[par] = (m, s2, t3)

            # ---- a-level merge (DVE 2x, contiguous e/o planes) ----
            me, s2e, t3e = planes["e"]
            mo, s2o, t3o = planes["o"]
            p1 = pool.tile([128, HW2], f16, tag="mid", bufs=6, name=f"p1_{g}")
            tt(p1[:, :], me[:, :], mo[:, :], MAX)
            u1 = pool.tile([128, HW2], f16, tag="mid", bufs=6, name=f"u1_{g}")
            tt(u1[:, :], me[:, :], mo[:, :], MIN)
            u2 = pool.tile([128, HW2], f16, tag="mid", bufs=6, name=f"u2_{g}")
            tt(u2[:, :], s2e[:, :], s2o[:, :], MAX)
            p2 = pool.tile([128, HW2], f16, tag="mid", bufs=6, name=f"p2_{g}")
            tt(p2[:, :], u1[:, :], u2[:, :], MAX)
            w2 = pool.tile([128, HW2], f16, tag="mid", bufs=6, name=f"w2_{g}")
            tt(w2[:, :], me[:, :], s2o[:, :], MIN)
            w3 = pool.tile([128, HW2], f16, tag="mid", bufs=6, name=f"w3_{g}")
            tt(w3[:, :], s2e[:, :], mo[:, :], MIN)
            w4 = pool.tile([128, HW2], f16, tag="mid", bufs=6, name=f"w4_{g}")
            tt(w4[:, :], w2[:, :], w3[:, :], MAX)
            w1 = pool.tile([128, HW2], f16, tag="mid", bufs=6, name=f"w1_{g}")
            tt(w1[:, :], t3e[:, :], t3o[:, :], MAX)
            p3 = pool.tile([128, HW2], f16, tag="mid", bufs=6, name=f"p3_{g}")
            tt(p3[:, :], w1[:, :], w4[:, :], MAX)

            # ---- b-level: deint (scalar) + merge (DVE, fp16 2x) ----
            def deint(src, w_out, tag, name):
                v = src.rearrange("p (j two) -> p j two", two=2)
                te = pool.tile([128, w_out], f16, tag=tag, bufs=6, name=name + "e")
                nc.scalar.copy(te[:, :], v[:, :, 0])
                to = pool.tile([128, w_out], f16, tag=tag, bufs=6, name=name + "o")
                nc.scalar.copy(to[:, :], v[:, :, 1])
                return te, to

            p1e, p1o = deint(p1, NBW, "eob", f"p1_{g}")
            p2e, p2o = deint(p2, NBW, "eob", f"p2_{g}")
            p3e, p3o = deint(p3, NBW, "eob", f"p3_{g}")
            z1 = pool.tile([128, NBW], f16, tag="small", bufs=6, name=f"z1_{g}")
            tt(z1[:, :], p1e[:, :], p1o[:, :], MIN)
            z2 = pool.tile([128, NBW], f16, tag="small", bufs=6, name=f"z2_{g}")
            tt(z2[:, :], p2e[:, :], p2o[:, :], MAX)
            c2 = pool.tile([128, NBW], f16, tag="small", bufs=6, name=f"c2_{g}")
            tt(c2[:, :], z1[:, :], z2[:, :], MAX)
            z4 = pool.tile([128, NBW], f16, tag="small", bufs=6, name=f"z4_{g}")
            tt(z4[:, :], p1e[:, :], p2o[:, :], MIN)
            z5 = pool.tile([128, NBW], f16, tag="small", bufs=6, name=f"z5_{g}")
            tt(z5[:, :], p2e[:, :], p1o[:, :], MIN)
            z6 = pool.tile([128, NBW], f16, tag="small", bufs=6, name=f"z6_{g}")
            tt(z6[:, :], z4[:, :], z5[:, :], MAX)
            z3 = pool.tile([128, NBW], f16, tag="small", bufs=6, name=f"z3_{g}")
            tt(z3[:, :], p3e[:, :], p3o[:, :], MAX)
            c3 = pool.tile([128, NBW], f16, tag="small", bufs=6, name=f"c3_{g}")
            tt(c3[:, :], z3[:, :], z6[:, :], MAX)

            # ---- quantile accumulation ----
            nc.tensor.matmul(
                psum_q[:, :], lhsT=wema_sb[:, :], rhs=eq_g[:, :],
                start=True, stop=False,
            )
            nc.tensor.matmul(
                psum_q[:, :], lhsT=wq_sb[:, :], rhs=c2[:, :],
                start=False, stop=False,
            )
            nc.tensor.matmul(
                psum_q[:, :], lhsT=wq_sb[:, :], rhs=c3[:, :],
                start=False, stop=True,
            )

            # ---- tail: u = num'/den', expand x4, write ----
            rec = pool.tile([128, NBW], f32, tag="rec", bufs=2, name=f"rec_{g}")
            act_recip(rec[:, :], psum_s[:, :])
            u = pool.tile([128, NBW], f32, tag="u", bufs=2, name=f"u_{g}")
            tt(u[:, :], psum_q[:, :], rec[:, :], MULT)
            u_tiles[g] = u

        def emit_tail(g):
            u = u_tiles[g]
            u4 = pool.tile([128, W], f32, tag="u4", bufs=2, name=f"u4_{g}")
            u4v = u4.rearrange("p (j c) -> p j c", c=BS)
            nc.vector.tensor_copy(u4v[:, :, 0], u[:, :])
            for c in range(1, BS):
                nc.scalar.copy(u4v[:, :, c], u[:, :])

            for t in range(4):
                nc.gpsimd.dma_start(y6[g, t], u4[:, :])

        WRITE_DELAY = 0  # defer writes K subchunks (NSUB = fully phased)
        for g in range(NSUB):
            emit_compute(g)
            if g - WRITE_DELAY >= 0:
                emit_tail(g - WRITE_DELAY)
        for g in range(max(0, NSUB - WRITE_DELAY), NSUB):
            emit_tail(g)

    _split_multi_waits(nc)
    return nc


def _get_nc():
    if "nc" not in _CACHE:
        _CACHE["nc"] = _build()
    return _CACHE["nc"]


def kernel(current_errors, ema_errors, ema_quantile):
    from concourse.bass_utils import run_bass_kernel_spmd

    x = np.asarray(current_errors, dtype=np.float32).reshape(B, H, W)
    ee_full = np.asarray(ema_errors, dtype=np.float32).reshape(H // BS, W // BS)
    eq_full = np.asarray(ema_quantile, dtype=np.float32).reshape(H // BS, W // BS)

    # host-side EMA prep (inputs scaled by 100; u = num'/den' is unchanged)
    ee2 = (EMA_SCALE * ee_full + EMA_BIAS).astype(np.float16)
    eq2 = (EMA_SCALE * eq_full).astype(np.float16)

    p = np.arange(128)
    mask = (p[:, None] % IB) == ((p[None, :] % 32) // BS)
    wsum = np.where(mask, np.float16(W_SUM), np.float16(0.0))
    wq = np.where(mask, np.float16(W_Q), np.float16(0.0))
    pp = np.arange(IB)
    wema = np.where(
        pp[:, None] == ((p[None, :] % 32) // BS),
        np.float16(1.0), np.float16(0.0),
    )

    in_maps = []
    for k in range(NCORES):
        xs = np.ascontiguousarray(x[:, k * HS : (k + 1) * HS, :]).reshape(ROWS, W)
        ees = np.ascontiguousarray(ee2[k * NBH : (k + 1) * NBH, :])
        eqs = np.ascontiguousarray(eq2[k * NBH : (k + 1) * NBH, :])
        in_maps.append(
            {"x": xs, "ee": ees, "eq": eqs, "wsum": wsum, "wq": wq, "wema": wema}
        )

    nc = _get_nc()
    trace = bool(int(os.environ.get("KERNEL_TRACE", "0")))
    try:
        res = run_bass_kernel_spmd(
            nc, in_maps, core_ids=list(range(NCORES)), trace=trace
        )
    except Exception:
        # transient device state (e.g. NRT_EXEC_UNIT_UNRECOVERABLE) — retry once
        res = run_bass_kernel_spmd(
            nc, in_maps, core_ids=list(range(NCORES)), trace=trace
        )
    _CACHE["last_results"] = res

    out = np.empty((B, 1, H, W), dtype=np.float32)
    for k in range(NCORES):
        out[:, 0, k * HS : (k + 1) * HS, :] = res.results[k]["y"].reshape(B, HS, W)
    return out


# revision 14
# speedup vs baseline: 1.1725x; 1.1725x over previous
"""Trainium2 Bass kernel for BlockUncertaintyTracker (segment_reduce).

Computes, per 4x4 block of a [16,1,2048,2048] image batch:
  - mean over the 16 block elements, averaged over batch
  - 0.9-quantile (= 0.5*(2nd largest + 3rd largest)), averaged over batch
  - EMA update of both stats, then broadcasts the ratio back to full shape.

Sharding: spatial over H across 8 cores (256 image rows / 64 block rows per
core). Every core sees all 16 batch elements for its rows, so no collectives
are needed.

Row-major subchunk pipeline (8 subchunks/core, each = 8 block rows x all 16
batches, partition p = (b,i)): output writes of subchunk g overlap input
reads of g+1, so the DMA queues stream reads+writes continuously instead of
serializing a write-only tail.

Engine split per subchunk (4 row-phase tiles R_r [128,2048] f32):
  - ScalarE: cast -> fp16, even/odd de-interleaves, table Reciprocal,
    most of the output expand.
  - VectorE: vertical sorted-3 + a-level and b-level merges (all fp16
    contiguous step-1 = 2x mode) + ratio multiply.
  - GpSimd:  ALU unused (shares SBUF ports with DVE and its copies run
    ~3us; measured to degrade DVE to 1x). Only issues input-load DMAs so
    descriptor generation spreads across sequencers.
  - TensorE: block sums via 16 strided-rhs fp16 matmuls (1 cyc/row vs 4 for
    f32) + quantile + EMA terms accumulated in PSUM. The EMA buffers enter
    PSUM through two tiny matmuls of host-precomputed (99*ema + 100*eps)
    fp16 tensors: scaling num and den by 100 leaves the ratio unchanged and
    keeps every lhsT weight fp16-normal.
  - Output: u expanded x4 along columns on ScalarE, written as 4
    full-partition 1 MiB DMAs per subchunk (4 batches each).
"""

import os

import numpy as np

# ---- problem constants (hardcoded; kernel.py must be self-contained) ----
B = 16          # batch
H = 2048
W = 2048
BS = 4          # block size
NCORES = 8
HS = H // NCORES            # 256 rows per core per batch
NBH = HS // BS              # 64 block rows per core
NBW = W // BS               # 512 block cols
ROWS = B * HS               # 4096 rows in a per-core slab
NSUB = 8                    # subchunks per core
IB = NBH // NSUB            # 8 block rows per subchunk
DECAY = 0.99
ALPHA = 0.1
EPS = 1e-5
# num' = 100*num, den' = 100*den; u = num'/den' unchanged.
W_SUM = 1.0 / (BS * BS * B)       # 100*(1-decay)*mean weight = plain mean
W_Q = 0.5 / B                     # 100*(1-decay)*0.5/B = 0.03125 (fp16 exact)
EMA_SCALE = 100.0 * DECAY         # 99.0
EMA_BIAS = 100.0 * EPS            # 1e-3 (den only)

_CACHE = {}


def _split_multi_waits(nc):
    """This walrus build encodes at most ONE sync wait per instruction.
    Tile attaches several. Hoist excess waits onto same-engine NOPs placed
    immediately before the owning instruction (same engine stream => same
    semantics)."""
    import concourse.mybir as mybir

    plans = []  # (inst_name, extra_waits)
    for f in nc.m.functions:
        for bb in f.blocks:
            for inst in bb.instructions:
                si = getattr(inst, "sync_info", None)
                waits = list(si.on_wait) if (si and si.on_wait) else []
                if len(waits) > 1:
                    si.on_wait = [waits[-1]]
                    plans.append((inst.name, waits[:-1]))

    if not plans:
        return

    nop_for = {}
    stray = set()
    for iname, extra in plans:
        nops = []
        for w in extra:
            nop = nc.engines[nc.inst_map[iname].engine].nop(nofuse=True).ins
            nop.sync_info = mybir.SyncInfo(on_wait=[w], on_update=[])
            nops.append(nop)
            stray.add(nop.name)
        nop_for[iname] = nops

    for f in nc.m.functions:
        for bb in f.blocks:
            out = []
            changed = False
            for inst in bb.instructions:
                if inst.name in stray:
                    changed = True
                    continue
                if inst.name in nop_for:
                    out.extend(nop_for[inst.name])
                    changed = True
                out.append(inst)
            if changed:
                bb.instructions = out


def _build():
    """Builds the single-core Bass program (SPMD across 8 cores)."""
    from contextlib import ExitStack

    import concourse.bass as bass
    import concourse.mybir as mybir
    import concourse.tile as tile

    f32 = mybir.dt.float32
    f16 = mybir.dt.float16
    MAX = mybir.AluOpType.max
    MIN = mybir.AluOpType.min
    MULT = mybir.AluOpType.mult

    nc = bass.Bass("TRN2", target_bir_lowering=False, debug=False)

    x = nc.dram_tensor("x", [ROWS, W], f32, kind="ExternalInput").ap()
    # host-precomputed 99*ema_errors + 1e-3 / 99*ema_quantile, fp16
    ee = nc.dram_tensor("ee", [NBH, NBW], f16, kind="ExternalInput").ap()
    eq = nc.dram_tensor("eq", [NBH, NBW], f16, kind="ExternalInput").ap()
    # wsum[p, m] = W_SUM  iff p%8 == (m%32)//4   (batch fold + m duplication)
    # wq  [p, m] = W_Q    same mask
    # wema[p, m] = 1.0    iff p%8 == (m%32)//4   (p in local subchunk coords)
    wsum = nc.dram_tensor("wsum", [128, 128], f16, kind="ExternalInput").ap()
    wq = nc.dram_tensor("wq", [128, 128], f16, kind="ExternalInput").ap()
    wema = nc.dram_tensor("wema", [IB, 128], f16, kind="ExternalInput").ap()
    y = nc.dram_tensor("y", [ROWS, W], f32, kind="ExternalOutput").ap()

    # input row = b*256 + g*32 + i*4 + r; per (g, r): [16, 8, 2048] zips
    # against [128=(b,i), 2048]; 8 KiB lines sustain full DMA bandwidth
    xr = x.rearrange("(b g i r) w -> g r b i w", b=B, g=NSUB, i=IB, r=BS)
    # output row = (t*4+beta)*256 + g*32 + i*4 + r; per (g, t): [4, 8, 4, 2048]
    # zips against 128 partitions as (beta, i, r)
    y6 = y.rearrange(
        "(t beta g i r) w -> g t beta i r w", t=4, beta=4, g=NSUB, i=IB, r=BS
    )

    with tile.TileContext(nc) as tc, ExitStack() as ctx:
        pool = ctx.enter_context(tc.tile_pool(name="work", bufs=1))
        ppool = ctx.enter_context(tc.tile_pool(name="acc", bufs=1, space="PSUM"))

        wsum_sb = pool.tile([128, 128], f16, tag="wsum")
        nc.sync.dma_start(wsum_sb[:, :], wsum)
        wq_sb = pool.tile([128, 128], f16, tag="wq")
        nc.sync.dma_start(wq_sb[:, :], wq)
        wema_sb = pool.tile([IB, 128], f16, tag="wema")
        nc.sync.dma_start(wema_sb[:, :], wema)

        def tt(dst, a, bb, op):
            nc.vector.tensor_tensor(dst, a, bb, op)

        def act_recip(out, in_):
            # table-based reciprocal on the Activation engine (~2e-5 max
            # err on our range; bass's wrapper over-conservatively bans it)
            nc.scalar.add_instruction(
                mybir.InstActivation(
                    name=nc.get_next_instruction_name(),
                    func=mybir.ActivationFunctionType.Reciprocal,
                    ins=[
                        nc.scalar.lower_ap(in_),
                        mybir.ImmediateValue(dtype=f32, value=0.0),
                        mybir.ImmediateValue(dtype=f32, value=1.0),
                        mybir.ImmediateValue(dtype=f32, value=0.0),
                    ],
                    outs=[nc.scalar.lower_ap(out)],
                )
            )

        HW2 = W // 2
        u_tiles = {}
        front = {}

        def emit_front(g):
            """Loads + casts + sum matmuls (scalar/PE/DMA side)."""
            rt = pool.tile([128, BS * W], f32, tag="raw", bufs=2, name=f"rt_{g}")
            rtv = rt.rearrange("p (r w) -> p r w", r=BS)
            for r in range(BS):
                nc.gpsimd.dma_start(rtv[:, r, :], xr[g, r])

            psum_s = ppool.tile([128, NBW], f32, tag="ps", bufs=2, name=f"ps_{g}")
            psum_q = ppool.tile([128, NBW], f32, tag="pq", bufs=2, name=f"pq_{g}")

            ee_g = pool.tile([IB, NBW], f16, tag="eesb", bufs=2, name=f"ee_{g}")
            nc.gpsimd.dma_start(ee_g[:, :], ee[g * IB : (g + 1) * IB, :])
            eq_g = pool.tile([IB, NBW], f16, tag="eqsb", bufs=2, name=f"eq_{g}")
            nc.gpsimd.dma_start(eq_g[:, :], eq[g * IB : (g + 1) * IB, :])
            nc.tensor.matmul(
                psum_s[:, :], lhsT=wema_sb[:, :], rhs=ee_g[:, :],
                start=True, stop=False,
            )
            nc.tensor.matmul(
                psum_q[:, :], lhsT=wema_sb[:, :], rhs=eq_g[:, :],
                start=True, stop=False,
            )

            # fused cast + even/odd deinterleave (scalar):
            # bte[p, (r, j, ce)] = f16(x[p, r, 4j + 2ce]);  bto: odd cols
            rt4 = rt.rearrange("p (r j two) -> p r j two", r=BS, two=2)
            bte = pool.tile([128, BS * HW2], f16, tag="bte", bufs=2, name=f"bte_{g}")
            bto = pool.tile([128, BS * HW2], f16, tag="bto", bufs=2, name=f"bto_{g}")
            for r in range(BS):
                nc.scalar.copy(bte[:, r * HW2 : (r + 1) * HW2], rt4[:, r, :, 0])
                nc.scalar.copy(bto[:, r * HW2 : (r + 1) * HW2], rt4[:, r, :, 1])

            # block sums: 16 strided fp16 matmuls
            bqe = bte.rearrange("p (r j c) -> p r j c", r=BS, c=2)
            bqo = bto.rearrange("p (r j c) -> p r j c", r=BS, c=2)
            for r in range(BS):
                for bq, c in ((bqe, 0), (bqe, 1), (bqo, 0), (bqo, 1)):
                    nc.tensor.matmul(
                        psum_s[:, :], lhsT=wsum_sb[:, :], rhs=bq[:, r, :, c],
                        start=False,
                        stop=(r == BS - 1 and bq is bqo and c == 1),
                    )
            front[g] = (psum_s, psum_q, bte, bto)

        def emit_back(g):
            """Sort network + quantile matmuls + ratio (DVE/scalar/PE)."""
            psum_s, psum_q, bte, bto = front.pop(g)
            btev = bte.rearrange("p (r w) -> p r w", r=BS)
            btov = bto.rearrange("p (r w) -> p r w", r=BS)

            # vertical sorted-3 (DVE, fp16 2x) on e/o planes
            planes = {}
            for par, bv in (("e", btev), ("o", btov)):
                b0, b1, b2_, b3 = (bv[:, r, :] for r in range(BS))
                v1 = pool.tile([128, HW2], f16, tag="big", bufs=14, name=f"v1{par}_{g}")
                tt(v1[:, :], b0, b1, MAX)
                w1v = pool.tile([128, HW2], f16, tag="big", bufs=14, name=f"w1v{par}_{g}")
                tt(w1v[:, :], b0, b1, MIN)
                v2 = pool.tile([128, HW2], f16, tag="big", bufs=14, name=f"v2{par}_{g}")
                tt(v2[:, :], b2_, b3, MAX)
                w2v = pool.tile([128, HW2], f16, tag="big", bufs=14, name=f"w2v{par}_{g}")
                tt(w2v[:, :], b2_, b3, MIN)
                m = pool.tile([128, HW2], f16, tag="big", bufs=14, name=f"m{par}_{g}")
                tt(m[:, :], v1[:, :], v2[:, :], MAX)
                t1 = pool.tile([128, HW2], f16, tag="big", bufs=14, name=f"t1{par}_{g}")
                tt(t1[:, :], v1[:, :], v2[:, :], MIN)
                t2 = pool.tile([128, HW2], f16, tag="big", bufs=14, name=f"t2{par}_{g}")
                tt(t2[:, :], w1v[:, :], w2v[:, :], MAX)
                s2 = pool.tile([128, HW2], f16, tag="big", bufs=14, name=f"s2{par}_{g}")
                tt(s2[:, :], t1[:, :], t2[:, :], MAX)
                t3 = pool.tile([128, HW2], f16, tag="big", bufs=14, name=f"t3{par}_{g}")
                tt(t3[:, :], t1[:, :], t2[:, :], MIN)
                planes[par] = (m, s2, t3)

            # a-level merge (DVE 2x, contiguous e/o planes)
            me, s2e, t3e = planes["e"]
            mo, s2o, t3o = planes["o"]
            p1 = pool.tile([128, HW2], f16, tag="mid", bufs=10, name=f"p1_{g}")
            tt(p1[:, :], me[:, :], mo[:, :], MAX)
            u1 = pool.tile([128, HW2], f16, tag="mid", bufs=10, name=f"u1_{g}")
            tt(u1[:, :], me[:, :], mo[:, :], MIN)
            u2 = pool.tile([128, HW2], f16, tag="mid", bufs=10, name=f"u2_{g}")
            tt(u2[:, :], s2e[:, :], s2o[:, :], MAX)
            p2 = pool.tile([128, HW2], f16, tag="mid", bufs=10, name=f"p2_{g}")
            tt(p2[:, :], u1[:, :], u2[:, :], MAX)
            w2 = pool.tile([128, HW2], f16, tag="mid", bufs=10, name=f"w2_{g}")
            tt(w2[:, :], me[:, :], s2o[:, :], MIN)
            w3 = pool.tile([128, HW2], f16, tag="mid", bufs=10, name=f"w3_{g}")
            tt(w3[:, :], s2e[:, :], mo[:, :], MIN)
            w4 = pool.tile([128, HW2], f16, tag="mid", bufs=10, name=f"w4_{g}")
            tt(w4[:, :], w2[:, :], w3[:, :], MAX)
            w1 = pool.tile([128, HW2], f16, tag="mid", bufs=10, name=f"w1_{g}")
            tt(w1[:, :], t3e[:, :], t3o[:, :], MAX)
            p3 = pool.tile([128, HW2], f16, tag="mid", bufs=10, name=f"p3_{g}")
            tt(p3[:, :], w1[:, :], w4[:, :], MAX)

            # b-level: deint (scalar) + merge (DVE, fp16 2x)
            def deint(src, w_out, tag, name):
                v = src.rearrange("p (j two) -> p j two", two=2)
                te = pool.tile([128, w_out], f16, tag=tag, bufs=6, name=name + "e")
                nc.scalar.copy(te[:, :], v[:, :, 0])
                to = pool.tile([128, w_out], f16, tag=tag, bufs=6, name=name + "o")
                nc.scalar.copy(to[:, :], v[:, :, 1])
                return te, to

            p1e, p1o = deint(p1, NBW, "eob", f"p1_{g}")
            p2e, p2o = deint(p2, NBW, "eob", f"p2_{g}")
            p3e, p3o = deint(p3, NBW, "eob", f"p3_{g}")
            z1 = pool.tile([128, NBW], f16, tag="small", bufs=8, name=f"z1_{g}")
            tt(z1[:, :], p1e[:, :], p1o[:, :], MIN)
            z2 = pool.tile([128, NBW], f16, tag="small", bufs=8, name=f"z2_{g}")
            tt(z2[:, :], p2e[:, :], p2o[:, :], MAX)
            c2 = pool.tile([128, NBW], f16, tag="small", bufs=8, name=f"c2_{g}")
            tt(c2[:, :], z1[:, :], z2[:, :], MAX)
            z4 = pool.tile([128, NBW], f16, tag="small", bufs=8, name=f"z4_{g}")
            tt(z4[:, :], p1e[:, :], p2o[:, :], MIN)
            z5 = pool.tile([128, NBW], f16, tag="small", bufs=8, name=f"z5_{g}")
            tt(z5[:, :], p2e[:, :], p1o[:, :], MIN)
            z6 = pool.tile([128, NBW], f16, tag="small", bufs=8, name=f"z6_{g}")
            tt(z6[:, :], z4[:, :], z5[:, :], MAX)
            z3 = pool.tile([128, NBW], f16, tag="small", bufs=8, name=f"z3_{g}")
            tt(z3[:, :], p3e[:, :], p3o[:, :], MAX)
            c3 = pool.tile([128, NBW], f16, tag="small", bufs=8, name=f"c3_{g}")
            tt(c3[:, :], z3[:, :], z6[:, :], MAX)

            nc.tensor.matmul(
                psum_q[:, :], lhsT=wq_sb[:, :], rhs=c2[:, :],
                start=False, stop=False,
            )
            nc.tensor.matmul(
                psum_q[:, :], lhsT=wq_sb[:, :], rhs=c3[:, :],
                start=False, stop=True,
            )

            rec = pool.tile([128, NBW], f32, tag="rec", bufs=2, name=f"rec_{g}")
            act_recip(rec[:, :], psum_s[:, :])
            u = pool.tile([128, NBW], f32, tag="u", bufs=2, name=f"u_{g}")
            tt(u[:, :], psum_q[:, :], rec[:, :], MULT)
            u_tiles[g] = u

        def emit_tail(g):
            u = u_tiles.pop(g)
            u4 = pool.tile([128, W], f32, tag="u4", bufs=2, name=f"u4_{g}")
            u4v = u4.rearrange("p (j c) -> p j c", c=BS)
            nc.vector.tensor_copy(u4v[:, :, 0], u[:, :])
            for c in range(1, BS):
                nc.scalar.copy(u4v[:, :, c], u[:, :])

            for t in range(4):
                nc.gpsimd.dma_start(y6[g, t], u4[:, :])

        # software-pipelined emission: front(g+1) is queued before back(g)
        # so no engine queue head-blocks on a same-subchunk cross-engine dep
        for g in range(NSUB):
            emit_front(g)
            if g >= 1:
                emit_back(g - 1)
                emit_tail(g - 1)
        emit_back(NSUB - 1)
        emit_tail(NSUB - 1)

    _split_multi_waits(nc)
    return nc


def _get_nc():
    if "nc" not in _CACHE:
        _CACHE["nc"] = _build()
    return _CACHE["nc"]


def kernel(current_errors, ema_errors, ema_quantile):
    from concourse.bass_utils import run_bass_kernel_spmd

    x = np.asarray(current_errors, dtype=np.float32).reshape(B, H, W)
    ee_full = np.asarray(ema_errors, dtype=np.float32).reshape(H // BS, W // BS)
    eq_full = np.asarray(ema_quantile, dtype=np.float32).reshape(H // BS, W // BS)

    # host-side EMA prep (inputs scaled by 100; u = num'/den' is unchanged)
    ee2 = (EMA_SCALE * ee_full + EMA_BIAS).astype(np.float16)
    eq2 = (EMA_SCALE * eq_full).astype(np.float16)

    p = np.arange(128)
    mask = (p[:, None] % IB) == ((p[None, :] % 32) // BS)
    wsum = np.where(mask, np.float16(W_SUM), np.float16(0.0))
    wq = np.where(mask, np.float16(W_Q), np.float16(0.0))
    pp = np.arange(IB)
    wema = np.where(
        pp[:, None] == ((p[None, :] % 32) // BS),
        np.float16(1.0), np.float16(0.0),
    )

    in_maps = []
    for k in range(NCORES):
        xs = np.ascontiguousarray(x[:, k * HS : (k + 1) * HS, :]).reshape(ROWS, W)
        ees = np.ascontiguousarray(ee2[k * NBH : (k + 1) * NBH, :])
        eqs = np.ascontiguousarray(eq2[k * NBH : (k + 1) * NBH, :])
        in_maps.append(
            {"x": xs, "ee": ees, "eq": eqs, "wsum": wsum, "wq": wq, "wema": wema}
        )

    nc = _get_nc()
    trace = bool(int(os.environ.get("KERNEL_TRACE", "0")))
    try:
        res = run_bass_kernel_spmd(
            nc, in_maps, core_ids=list(range(NCORES)), trace=trace
        )
    except Exception:
        # transient device state (e.g. NRT_EXEC_UNIT_UNRECOVERABLE) — retry once
        res = run_bass_kernel_spmd(
            nc, in_maps, core_ids=list(range(NCORES)), trace=trace
        )
    _CACHE["last_results"] = res

    out = np.empty((B, 1, H, W), dtype=np.float32)
    for k in range(NCORES):
        out[:, 0, k * HS : (k + 1) * HS, :] = res.results[k]["y"].reshape(B, HS, W)
    return out
